# revision 1
# baseline (speedup 1.0000x reference)
"""CRF loss (forward-algorithm partition function minus gold path score, batch mean)
on 8 Trainium2 NeuronCores, data-parallel over the batch dimension.

Layout / algorithm notes
------------------------
Per core shard: 512 batches = 4 groups x 128 batch-columns.
Scan state alphaT [128 part = (group, tag), 128 free = batch col] in bf16.

Forward algorithm runs in exp-space:
    alpha_{s} = (Mblk^T @ alpha_{s-1}) * exp(em_s - MU)
with Mblk = blockdiag(exp(transitions)) so one PE matmul advances all 4
groups.  MU pre-scales away the mean per-step growth; every W steps alpha is
rescaled per-batch (scale broadcast via a PE matmul) and the log of the scale
is accumulated so logZ is exact.

The output is a scalar mean, so gold scores are only needed summed over the
batch.  Sum_{b,s} em[b,s,tag] and sum_{b,s} T[tag_{s-1},tag_s] are computed as
traces of PSUM-accumulated matmuls over fp8 one-hot tiles (4 timesteps packed
per matmul; diagonal 32x32 blocks extracted with a mask at the end).
"""

import numpy as np
import ml_dtypes

B, S, T = 4096, 512, 32
NCORES = 8
BS = B // NCORES          # batches per core
G, BG = 4, 128            # groups x batch-columns (G*BG == BS)
P = 128
NQ = S // 4               # gold quads per batch tile
QCHUNK = 32               # quads per gold DMA chunk
SCHUNK = 64               # scan steps per emission DMA chunk
W = 32                    # rescale interval
RESCALE_LAG = 6           # steps between computing a rescale and applying it
MU = float(np.log(T) + 1.0)

BF16 = ml_dtypes.bfloat16
FP8 = ml_dtypes.float8_e4m3

_GRAPH = None

import os
DBG_NO_GOLD = bool(int(os.environ.get("CRF_DBG_NO_GOLD", "0")))
DBG_NO_SCAN = bool(int(os.environ.get("CRF_DBG_NO_SCAN", "0")))
DBG_NO_RESCALE = bool(int(os.environ.get("CRF_DBG_NO_RESCALE", "0")))
DBG_STEPS = int(os.environ.get("CRF_DBG_STEPS", "0"))


def _build_graph():
    from concourse import bacc, mybir, tile

    f32 = mybir.dt.float32
    bf16 = mybir.dt.bfloat16
    f8 = mybir.dt.float8e4
    Af = mybir.ActivationFunctionType
    Op = mybir.AluOpType
    AX = mybir.AxisListType.X

    nc = bacc.Bacc(
        "TRN2",
        target_bir_lowering=False,
        debug=False,
        enable_asserts=False,
        num_devices=NCORES,
    )

    em_scan = nc.dram_tensor("em_scan", [P, S * BG], bf16, kind="ExternalInput")
    gold_in = nc.dram_tensor("gold_in", [G, P, NQ * 288], f8, kind="ExternalInput")
    trep = nc.dram_tensor("trep", [P, T], f32, kind="ExternalInput")
    gmask = nc.dram_tensor("gmask", [P, 256], f32, kind="ExternalInput")
    svec = nc.dram_tensor("svec", [P, 1], f32, kind="ExternalInput")
    evec = nc.dram_tensor("evec", [P, 1], f32, kind="ExternalInput")
    sel4 = nc.dram_tensor("sel4", [G, P], bf16, kind="ExternalInput")
    bones4 = nc.dram_tensor("bones4", [P, G], bf16, kind="ExternalInput")
    ones8 = nc.dram_tensor("ones8", [P, 1], f8, kind="ExternalInput")
    ones32 = nc.dram_tensor("ones32", [P, 1], f32, kind="ExternalInput")
    out = nc.dram_tensor("out", [1, 8], f32, kind="ExternalOutput")

    em_ap = em_scan.ap()
    gold_ap = gold_in.ap()

    with tile.TileContext(nc) as tc:
        with (
            tc.tile_pool(name="cpool", bufs=1) as cpool,
            tc.tile_pool(name="empool", bufs=2) as empool,
            tc.tile_pool(name="xppool", bufs=2) as xppool,
            tc.tile_pool(name="gpool", bufs=2) as gpool,
            tc.tile_pool(name="apool", bufs=3) as apool,
            tc.tile_pool(name="rpool", bufs=2) as rpool,
            tc.tile_pool(name="pspool", bufs=2, space="PSUM") as pspool,
            tc.tile_pool(name="psg", bufs=1, space="PSUM") as psgpool,
        ):
            # ---- constants ----
            trep_t = cpool.tile([P, T], f32)
            nc.sync.dma_start(out=trep_t[:], in_=trep.ap())
            gmask_t = cpool.tile([P, 256], f32)
            nc.sync.dma_start(out=gmask_t[:], in_=gmask.ap())
            sv_t = cpool.tile([P, 1], f32)
            nc.sync.dma_start(out=sv_t[:], in_=svec.ap())
            ev_t = cpool.tile([P, 1], f32)
            nc.sync.dma_start(out=ev_t[:], in_=evec.ap())
            sel4_t = cpool.tile([G, P], bf16)
            nc.sync.dma_start(out=sel4_t[:], in_=sel4.ap())
            bones4_t = cpool.tile([P, G], bf16)
            nc.sync.dma_start(out=bones4_t[:], in_=bones4.ap())
            ones8_t = cpool.tile([P, 1], f8)
            nc.sync.dma_start(out=ones8_t[:], in_=ones8.ap())
            ones32_t = cpool.tile([P, 1], f32)
            nc.sync.dma_start(out=ones32_t[:], in_=ones32.ap())

            es_t = cpool.tile([P, 1], f32)
            nc.scalar.activation(es_t[:], sv_t[:], Af.Exp)
            ee_t = cpool.tile([P, 1], f32)
            nc.scalar.activation(ee_t[:], ev_t[:], Af.Exp)

            mexp_t = cpool.tile([P, T], bf16)
            nc.scalar.activation(mexp_t[:], trep_t[:], Af.Exp)
            mblk_t = cpool.tile([P, P], bf16)
            nc.vector.memset(mblk_t[:], 0.0)
            for g in range(G):
                nc.vector.tensor_copy(
                    mblk_t[g * 32 : (g + 1) * 32, g * 32 : (g + 1) * 32],
                    mexp_t[g * 32 : (g + 1) * 32, :],
                )

            negmu_t = cpool.tile([P, 1], f32)
            nc.vector.memset(negmu_t[:], -MU)
            logz4_t = cpool.tile([G, BG], f32)
            nc.vector.memset(logz4_t[:], 0.0)
            finals_t = cpool.tile([P, 8], f32)
            nc.vector.memset(finals_t[:], 0.0)
            ttr_out_t = cpool.tile([P, 256], f32)

            # ---- gold psum accumulators ----
            goldps_full = psgpool.tile([P, 256], f32, name="goldps_full")
            goldps = goldps_full[:]
            DBG_NO_COUNTS = bool(int(os.environ.get("CRF_DBG_NO_COUNTS", "0")))
            c0ps_t = psgpool.tile([32, 1], f32, tag="c0ps", name="c0ps_t")
            cLps_t = psgpool.tile([32, 1], f32, tag="cLps", name="cLps_t")
            c0ps = c0ps_t[:]
            cLps = cLps_t[:]

            # generator of gold-side operations, interleaved into the scan
            def gold_op_stream():
                first = True
                for qc in range(S // (4 * QCHUNK)):  # 4 quad-chunks
                    for g in range(G):
                        gt = gpool.tile([P, QCHUNK * 288], f8, name="goldtile")
                        lo = qc * QCHUNK * 288
                        nc.sync.dma_start(
                            out=gt[:], in_=gold_ap[g, :, lo : lo + QCHUNK * 288]
                        )
                        for j in range(QCHUNK):
                            lhsT = gt[:, j * 288 + 160 : j * 288 + 288]
                            rhs = gt[:, j * 288 : j * 288 + 256]
                            last = (qc == 3) and (g == G - 1) and (j == QCHUNK - 1)
                            nc.tensor.matmul(
                                goldps,
                                lhsT=lhsT,
                                rhs=rhs,
                                start=first,
                                stop=last,
                                skip_group_check=True,
                            )
                            first = False
                            if qc == 0 and j == 0 and not DBG_NO_COUNTS:
                                # count of tag at s=0 (start-transition term)
                                nc.tensor.matmul(
                                    c0ps,
                                    lhsT=gt[:, 160:192],
                                    rhs=ones8_t[:],
                                    start=(g == 0),
                                    stop=(g == G - 1),
                                    skip_group_check=True,
                                )
                            if qc == 3 and j == QCHUNK - 1 and not DBG_NO_COUNTS:
                                # count of tag at s=S-1 (end-transition term)
                                nc.tensor.matmul(
                                    cLps,
                                    lhsT=gt[:, j * 288 + 256 : j * 288 + 288],
                                    rhs=ones8_t[:],
                                    start=(g == 0),
                                    stop=(g == G - 1),
                                    skip_group_check=True,
                                )
                            yield

            gold_ops = gold_op_stream() if not DBG_NO_GOLD else iter(())

            # ---- scan chunk 0 + alpha0 ----
            em_t = empool.tile([P, SCHUNK * BG], bf16, name="emchunk")
            nc.sync.dma_start(out=em_t[:], in_=em_ap[:, 0 : SCHUNK * BG])
            xp_t = xppool.tile([P, SCHUNK * BG], bf16, name="xpchunk")
            nc.scalar.activation(xp_t[:], em_t[:], Af.Exp, bias=negmu_t[:])

            alpha = apool.tile([P, BG], bf16, tag="alpha", name="alpha")
            nc.vector.tensor_scalar_mul(alpha[:], xp_t[:, 0:BG], es_t[:])
            pending_bc = None
            pending_apply_s = -1

            # ---- main scan ----
            nsteps = DBG_STEPS if DBG_STEPS else S
            for s in range(1, nsteps):
                c, so = divmod(s, SCHUNK)
                if so == 0:
                    em_t = empool.tile([P, SCHUNK * BG], bf16, name="emchunk")
                    nc.sync.dma_start(
                        out=em_t[:],
                        in_=em_ap[:, c * SCHUNK * BG : (c + 1) * SCHUNK * BG],
                    )
                    xp_t = xppool.tile([P, SCHUNK * BG], bf16, name="xpchunk")
                    nc.scalar.activation(xp_t[:], em_t[:], Af.Exp, bias=negmu_t[:])

                if DBG_NO_SCAN:
                    next(gold_ops, None)
                    next(gold_ops, None)
                    continue
                ps = pspool.tile([P, BG], f32, tag="scanps", name="scanps")
                nc.tensor.matmul(ps[:], lhsT=mblk_t[:], rhs=alpha[:], start=True, stop=True)

                # keep the gold matmul pipeline fed (1 quad per scan step)
                next(gold_ops, None)
                if s == 1:
                    next(gold_ops, None)

                alpha_new = apool.tile([P, BG], bf16, tag="alpha", name="alpha")
                nc.vector.tensor_tensor(
                    alpha_new[:], ps[:], xp_t[:, so * BG : (so + 1) * BG], Op.mult
                )
                alpha = alpha_new

                if pending_bc is not None and s == pending_apply_s:
                    alpha_rs = apool.tile([P, BG], bf16, tag="alpha", name="alpha")
                    nc.vector.tensor_tensor(alpha_rs[:], pending_bc[:], alpha[:], Op.mult)
                    alpha = alpha_rs
                    pending_bc = None

                if s % W == 0 and s <= S - W and not DBG_NO_RESCALE:
                    # per-batch rescale: group mass via PE column-sum, 1/c via
                    # ACT exp(-ln(c)); applied RESCALE_LAG steps later so the
                    # chain overlaps the scan.
                    cps = pspool.tile([G, BG], f32, tag="cps", name="cps", bufs=1)
                    nc.tensor.matmul(
                        cps[:], lhsT=bones4_t[:], rhs=alpha[:], start=True, stop=True
                    )
                    logc_t = rpool.tile([G, BG], f32, tag="logc", name="logc")
                    nc.scalar.activation(logc_t[:], cps[:], Af.Ln)
                    r4_t = rpool.tile([G, BG], bf16, tag="r4", name="r4")
                    with nc.allow_low_precision(
                        reason="bf16 rescale factor; its exact log is accumulated"
                    ):
                        nc.scalar.activation(r4_t[:], logc_t[:], Af.Exp, scale=-1.0)
                    lnr_t = rpool.tile([G, BG], f32, tag="lnr", name="lnr")
                    nc.scalar.activation(lnr_t[:], r4_t[:], Af.Ln)
                    nc.vector.tensor_tensor(
                        logz4_t[:], logz4_t[:], lnr_t[:], Op.subtract
                    )
                    bc = pspool.tile([P, BG], f32, tag="bcps", name="bcps", bufs=1)
                    nc.tensor.matmul(
                        bc[:], lhsT=sel4_t[:], rhs=r4_t[:], start=True, stop=True
                    )
                    pending_bc = bc
                    pending_apply_s = s + RESCALE_LAG

            # drain any remaining gold matmuls
            for _ in gold_ops:
                pass

            # ---- finalize forward: logZ = sum(logs) + log(sum_t alpha*exp(end)) ----
            aend = apool.tile([P, BG], bf16, tag="alpha", name="alpha")
            nc.vector.tensor_scalar_mul(aend[:], alpha[:], ee_t[:])
            gs = pspool.tile([G, BG], f32, tag="cps", name="gsps", bufs=1)
            nc.tensor.matmul(gs[:], lhsT=bones4_t[:], rhs=aend[:], start=True, stop=True)
            lngs_t = rpool.tile([G, BG], f32, tag="logc", name="lngs")
            nc.scalar.activation(lngs_t[:], gs[:], Af.Ln)
            nc.vector.tensor_tensor(logz4_t[:], logz4_t[:], lngs_t[:], Op.add)
            nc.vector.reduce_sum(finals_t[0:G, 0:1], logz4_t[:], axis=AX)

            # ---- finalize gold ----
            if not DBG_NO_GOLD:
                nc.vector.tensor_tensor(ttr_out_t[:], goldps, gmask_t[:], Op.mult)
                nc.vector.reduce_sum(finals_t[:, 1:2], ttr_out_t[:], axis=AX)
                if not DBG_NO_COUNTS:
                    nc.vector.tensor_tensor(finals_t[0:32, 2:3], c0ps, sv_t[0:32, :], Op.mult)
                    nc.vector.tensor_tensor(finals_t[0:32, 3:4], cLps, ev_t[0:32, :], Op.mult)

            # ---- partition-reduce the finals and write out ----
            finps = pspool.tile([1, 8], f32, tag="finps", name="finps", bufs=1)
            nc.tensor.matmul(
                finps[:], lhsT=ones32_t[:], rhs=finals_t[:], start=True, stop=True
            )
            outsb = cpool.tile([1, 8], f32)
            nc.vector.tensor_copy(outsb[:], finps[:])
            nc.sync.dma_start(out=out.ap(), in_=outsb[:])

    nc.compile()
    return nc


def _get_graph():
    global _GRAPH
    if _GRAPH is None:
        _GRAPH = _build_graph()
    return _GRAPH


def _host_inputs(transitions, start_transitions, end_transitions):
    """Constant / parameter-layout tensors shared by all cores."""
    Tm = np.asarray(transitions, np.float32)
    sv = np.asarray(start_transitions, np.float32)
    ev = np.asarray(end_transitions, np.float32)

    gmask = np.zeros((P, 256), np.float32)
    gmask[:, :P] = np.eye(P, dtype=np.float32)
    for j in range(4):
        gmask[j * 32 : (j + 1) * 32, P + j * 32 : P + (j + 1) * 32] = Tm.T

    trep = np.tile(Tm, (G, 1))
    svec = np.tile(sv, G)[:, None].astype(np.float32)
    evec = np.tile(ev, G)[:, None].astype(np.float32)

    k = np.arange(P)
    sel4 = (np.arange(G)[:, None] == (k[None, :] // 32)).astype(BF16)  # [G, P]
    bones4 = (np.arange(G)[None, :] == (k[:, None] // 32)).astype(BF16)  # [P, G]

    return {
        "trep": np.ascontiguousarray(trep),
        "gmask": gmask,
        "svec": svec,
        "evec": evec,
        "sel4": np.ascontiguousarray(sel4),
        "bones4": np.ascontiguousarray(bones4),
        "ones8": np.ones((P, 1), FP8),
        "ones32": np.ones((P, 1), np.float32),
    }


def _shard_inputs(emissions, tags, core):
    """Per-core data tensors: scan-layout emissions and gold fp8 quads."""
    bsl = slice(core * BS, (core + 1) * BS)
    em = np.asarray(emissions[bsl], np.float32)  # [BS, S, T]
    tg = np.asarray(tags[bsl]).astype(np.int64)  # [BS, S]

    # scan layout: [(g,t) partition, (s,b) free]
    em4 = em.reshape(G, BG, S, T)
    em_scan = (
        em4.transpose(0, 3, 2, 1).reshape(P, S * BG).astype(BF16)
    )  # [(g,t), (s,b)]

    # gold layout: per batch-tile, per quad q: 288 cols =
    #   [em_{4q}..em_{4q+3} (128) | oh_{4q-1}..oh_{4q+3} (160)]
    embt = em.reshape(G, BG, NQ, 4 * T).astype(FP8)  # em quads
    tg4 = tg.reshape(G, BG, S)
    oh = np.zeros((G, BG, S + 1, T), FP8)
    gi = np.arange(G)[:, None, None]
    bi = np.arange(BG)[None, :, None]
    si = np.arange(S)[None, None, :]
    oh[gi, bi, si + 1, tg4] = FP8(1.0)
    widx = (np.arange(NQ) * 4)[:, None] + np.arange(5)[None, :]  # [NQ, 5]
    oh_win = oh[:, :, widx, :].reshape(G, BG, NQ, 5 * T)

    goldarr = np.empty((G, BG, NQ, 288), FP8)
    goldarr[..., :128] = embt
    goldarr[..., 128:] = oh_win
    gold = goldarr.reshape(G, BG, NQ * 288)
    return {"em_scan": em_scan, "gold_in": np.ascontiguousarray(gold)}


def _numpy_reference(emissions, tags, mask, transitions, start_transitions, end_transitions):
    """Slow numpy fallback, only used if mask is not all ones."""
    em = np.asarray(emissions, np.float64)
    tg = np.asarray(tags).astype(np.int64)
    mk = np.asarray(mask).astype(bool)
    Tm = np.asarray(transitions, np.float64)
    sv = np.asarray(start_transitions, np.float64)
    ev = np.asarray(end_transitions, np.float64)
    Bn, Sn, Tn = em.shape

    t0 = tg[:, 0]
    score = sv[t0] + np.take_along_axis(em[:, 0], t0[:, None], axis=1)[:, 0]
    maskf = mk[:, 1:].astype(np.float64)
    trans_sc = Tm[tg[:, :-1], tg[:, 1:]]
    emit_sc = np.take_along_axis(em[:, 1:], tg[:, 1:, None], axis=2)[..., 0]
    gold = score + ((trans_sc + emit_sc) * maskf).sum(axis=1)
    last_idx = mk.sum(axis=1).astype(np.int64) - 1
    last_tags = np.take_along_axis(tg, last_idx[:, None], axis=1)[:, 0]
    gold = gold + ev[last_tags]

    sc = sv[None, :] + em[:, 0]
    for s in range(1, Sn):
        nxt = sc[:, :, None] + Tm[None] + em[:, s][:, None, :]
        m = nxt.max(axis=1)
        nxt = m + np.log(np.exp(nxt - m[:, None, :]).sum(axis=1))
        sc = np.where(mk[:, s][:, None], nxt, sc)
    sc = sc + ev[None, :]
    m = sc.max(axis=1)
    fwd = m + np.log(np.exp(sc - m[:, None]).sum(axis=1))
    return np.array((fwd - gold).mean(), np.float32)


def kernel(emissions, tags, mask, transitions, start_transitions, end_transitions,
           _want_results=False, _trace=False):
    emissions = np.asarray(emissions)
    tags = np.asarray(tags)
    mask = np.asarray(mask)

    if not np.asarray(mask).all():
        return _numpy_reference(
            emissions, tags, mask, transitions, start_transitions, end_transitions
        )

    from concourse.bass_utils import run_bass_kernel_spmd

    nc = _get_graph()
    shared = _host_inputs(transitions, start_transitions, end_transitions)
    in_maps = []
    for c in range(NCORES):
        m = dict(shared)
        m.update(_shard_inputs(emissions, tags, c))
        in_maps.append(m)

    res = run_bass_kernel_spmd(nc, in_maps, list(range(NCORES)), trace=_trace)

    tot_fwd = 0.0
    tot_gold = 0.0
    for c in range(NCORES):
        fin = np.asarray(res.results[c]["out"], np.float64)[0]
        tot_fwd += fin[0]
        tot_gold += fin[1] + fin[2] + fin[3]
    tot_fwd += B * S * MU
    loss = (tot_fwd - tot_gold) / B
    if _want_results:
        return np.array(loss, np.float32), res
    return np.array(loss, np.float32)



# revision 5
# speedup vs baseline: 2.2751x; 2.2751x over previous
"""CRF loss (forward-algorithm partition function minus gold path score, batch mean)
on 8 Trainium2 NeuronCores, data-parallel over the batch dimension.

Layout / algorithm notes
------------------------
Per core shard: 512 batches = 4 groups x 128 batch-columns.
State tiles [128 part = (group, tag), 128 free = batch col] in bf16.

The partition function runs as a BIDIRECTIONAL exp-space scan meeting in the
middle, which halves the serial matmul->multiply dependency chain:

    fwd:  alpha_s = (Mblk^T  @ alpha_{s-1}) * xp_s      s = 1..255
    bwd:  beta_s  = (MblkT^T @ beta_{s+1})  * xp_s      s = 510..256
    Z_b  = sum_t alpha_255[t,b] * (M beta_256)[t,b]

with xp_s = exp(em_s - MU), Mblk = blockdiag(exp(T)), MblkT =
blockdiag(exp(T)^T).  MU = log(T)+1 cancels the mean per-step growth, so the
state drifts only O(sqrt(S)) nats per batch and needs NO rescaling inside
fp32/bf16 exponent range; logZ = log(Z_b) + S*MU exactly.

The two chains are independent, so the tensor engine interleaves fwd/bwd
matmuls while the vector engine interleaves the emission multiplies: the
per-step serial latency is paid only 256 times instead of 512.

The gold path score is pure integer indexing on tags (gathers + bincounts)
plus one emission gather; it is computed on host in float64 (the device keeps
the O(B*S*T^2) forward algorithm).
"""

import numpy as np
import ml_dtypes

B, S, T = 4096, 512, 32
NCORES = 8
BS = B // NCORES          # batches per core
G, BG = 4, 128            # groups x batch-columns (G*BG == BS)
P = 128
HALF = S // 2             # steps per chain
SCHUNK = 32               # scan steps per emission DMA chunk
NCH = HALF // SCHUNK      # chunks per chain
MU = float(np.log(T) + 1.0)

BF16 = ml_dtypes.bfloat16

_GRAPH = None


def _build_graph():
    from concourse import bacc, mybir, tile

    f32 = mybir.dt.float32
    bf16 = mybir.dt.bfloat16
    Af = mybir.ActivationFunctionType
    Op = mybir.AluOpType
    AX = mybir.AxisListType.X

    nc = bacc.Bacc(
        "TRN2",
        target_bir_lowering=False,
        debug=False,
        enable_asserts=False,
        num_devices=NCORES,
    )

    em_scan = nc.dram_tensor("em_scan", [P, S * BG], bf16, kind="ExternalInput")
    trep = nc.dram_tensor("trep", [P, T], f32, kind="ExternalInput")
    trepT = nc.dram_tensor("trepT", [P, T], f32, kind="ExternalInput")
    svec = nc.dram_tensor("svec", [P, 1], f32, kind="ExternalInput")
    evec = nc.dram_tensor("evec", [P, 1], f32, kind="ExternalInput")
    bones4 = nc.dram_tensor("bones4", [P, G], bf16, kind="ExternalInput")
    onesG = nc.dram_tensor("onesG", [G, 1], f32, kind="ExternalInput")
    out = nc.dram_tensor("out", [1, 1], f32, kind="ExternalOutput")

    em_ap = em_scan.ap()
    CH = SCHUNK * BG          # free cols per chunk
    BWD0 = HALF * BG          # column offset of the backward stream

    with tile.TileContext(nc) as tc:
        with (
            tc.tile_pool(name="cpool", bufs=1) as cpool,
            tc.tile_pool(name="empf", bufs=2) as empf,
            tc.tile_pool(name="empb", bufs=2) as empb,
            tc.tile_pool(name="xpf", bufs=3) as xpfp,
            tc.tile_pool(name="xpb", bufs=3) as xpbp,
            tc.tile_pool(name="apool", bufs=3) as apool,
            tc.tile_pool(name="bpool", bufs=3) as bpool,
            tc.tile_pool(name="psf", bufs=2, space="PSUM") as psfp,
            tc.tile_pool(name="psb", bufs=2, space="PSUM") as psbp,
            tc.tile_pool(name="psx", bufs=1, space="PSUM") as psxp,
        ):
            # ---- constants ----
            trep_t = cpool.tile([P, T], f32)
            nc.sync.dma_start(out=trep_t[:], in_=trep.ap())
            trepT_t = cpool.tile([P, T], f32)
            nc.sync.dma_start(out=trepT_t[:], in_=trepT.ap())
            sv_t = cpool.tile([P, 1], f32)
            nc.sync.dma_start(out=sv_t[:], in_=svec.ap())
            ev_t = cpool.tile([P, 1], f32)
            nc.sync.dma_start(out=ev_t[:], in_=evec.ap())
            bones4_t = cpool.tile([P, G], bf16)
            nc.sync.dma_start(out=bones4_t[:], in_=bones4.ap())
            onesG_t = cpool.tile([G, 1], f32)
            nc.sync.dma_start(out=onesG_t[:], in_=onesG.ap())

            es_t = cpool.tile([P, 1], f32)
            nc.scalar.activation(es_t[:], sv_t[:], Af.Exp)
            ee_t = cpool.tile([P, 1], f32)
            nc.scalar.activation(ee_t[:], ev_t[:], Af.Exp)

            mexp_t = cpool.tile([P, T], bf16)
            nc.scalar.activation(mexp_t[:], trep_t[:], Af.Exp)
            mexpT_t = cpool.tile([P, T], bf16)
            nc.scalar.activation(mexpT_t[:], trepT_t[:], Af.Exp)
            mblk_t = cpool.tile([P, P], bf16)
            nc.vector.memset(mblk_t[:], 0.0)
            mblkT_t = cpool.tile([P, P], bf16)
            nc.vector.memset(mblkT_t[:], 0.0)
            for g in range(G):
                sl = slice(g * 32, (g + 1) * 32)
                nc.vector.tensor_copy(mblk_t[sl, sl], mexp_t[sl, :])
                nc.vector.tensor_copy(mblkT_t[sl, sl], mexpT_t[sl, :])

            negmu_t = cpool.tile([P, 1], f32)
            nc.vector.memset(negmu_t[:], -MU)

            # ---- emission chunk streams (double-buffered DMA + exp) ----
            def issue_chunk(c):
                emf_t = empf.tile([P, CH], bf16, name="emf")
                nc.sync.dma_start(out=emf_t[:], in_=em_ap[:, c * CH : (c + 1) * CH])
                xf_t = xpfp.tile([P, CH], bf16, name="xpf")
                nc.scalar.activation(xf_t[:], emf_t[:], Af.Exp, bias=negmu_t[:])
                emb_t = empb.tile([P, CH], bf16, name="emb")
                nc.sync.dma_start(
                    out=emb_t[:], in_=em_ap[:, BWD0 + c * CH : BWD0 + (c + 1) * CH]
                )
                xb_t = xpbp.tile([P, CH], bf16, name="xpb")
                nc.scalar.activation(xb_t[:], emb_t[:], Af.Exp, bias=negmu_t[:])
                return xf_t, xb_t

            xf_t, xb_t = issue_chunk(0)
            nxt = issue_chunk(1) if NCH > 1 else None

            # ---- init both chains (window 0) ----
            alpha = apool.tile([P, BG], bf16, tag="alpha", name="alpha")
            nc.vector.tensor_scalar_mul(alpha[:], xf_t[:, 0:BG], es_t[:])
            beta = bpool.tile([P, BG], bf16, tag="beta", name="beta")
            nc.vector.tensor_scalar_mul(beta[:], xb_t[:, 0:BG], ee_t[:])

            # ---- main bidirectional scan: windows 1..HALF-1 ----
            for w in range(1, HALF):
                c, so = divmod(w, SCHUNK)
                if so == 0:
                    xf_t, xb_t = nxt
                    nxt = issue_chunk(c + 1) if c + 1 < NCH else None

                psf = psfp.tile([P, BG], f32, tag="psf", name="psf")
                nc.tensor.matmul(psf[:], lhsT=mblk_t[:], rhs=alpha[:], start=True, stop=True)
                psb = psbp.tile([P, BG], f32, tag="psb", name="psb")
                nc.tensor.matmul(psb[:], lhsT=mblkT_t[:], rhs=beta[:], start=True, stop=True)

                alpha_new = apool.tile([P, BG], bf16, tag="alpha", name="alpha")
                nc.vector.tensor_tensor(
                    alpha_new[:], psf[:], xf_t[:, so * BG : (so + 1) * BG], Op.mult
                )
                alpha = alpha_new
                beta_new = bpool.tile([P, BG], bf16, tag="beta", name="beta")
                nc.vector.tensor_tensor(
                    beta_new[:], psb[:], xb_t[:, so * BG : (so + 1) * BG], Op.mult
                )
                beta = beta_new

            # ---- junction: Z = sum_t alpha_255 * (M beta_256) ----
            psj = psfp.tile([P, BG], f32, tag="psf", name="psj")
            nc.tensor.matmul(psj[:], lhsT=mblkT_t[:], rhs=beta[:], start=True, stop=True)
            zt = apool.tile([P, BG], bf16, tag="alpha", name="zt")
            nc.vector.tensor_tensor(zt[:], psj[:], alpha[:], Op.mult)

            gs = psxp.tile([G, BG], f32, tag="gs", name="gs")
            nc.tensor.matmul(gs[:], lhsT=bones4_t[:], rhs=zt[:], start=True, stop=True)
            lngs_t = cpool.tile([G, BG], f32)
            nc.scalar.activation(lngs_t[:], gs[:], Af.Ln)
            colsum_t = cpool.tile([G, 1], f32)
            nc.vector.reduce_sum(colsum_t[:], lngs_t[:], axis=AX)

            fin = psxp.tile([1, 1], f32, tag="fin", name="fin")
            nc.tensor.matmul(fin[:], lhsT=onesG_t[:], rhs=colsum_t[:], start=True, stop=True)
            outsb = cpool.tile([1, 1], f32)
            nc.vector.tensor_copy(outsb[:], fin[:])
            nc.sync.dma_start(out=out.ap(), in_=outsb[:])

    nc.compile()
    return nc


def _get_graph():
    global _GRAPH
    if _GRAPH is None:
        _GRAPH = _build_graph()
    return _GRAPH


def _host_inputs(transitions, start_transitions, end_transitions):
    """Constant / parameter-layout tensors shared by all cores."""
    Tm = np.asarray(transitions, np.float32)
    sv = np.asarray(start_transitions, np.float32)
    ev = np.asarray(end_transitions, np.float32)

    k = np.arange(P)
    bones4 = (np.arange(G)[None, :] == (k[:, None] // 32)).astype(BF16)  # [P, G]

    return {
        "trep": np.ascontiguousarray(np.tile(Tm, (G, 1))),
        "trepT": np.ascontiguousarray(np.tile(Tm.T, (G, 1))),
        "svec": np.tile(sv, G)[:, None].astype(np.float32),
        "evec": np.tile(ev, G)[:, None].astype(np.float32),
        "bones4": np.ascontiguousarray(bones4),
        "onesG": np.ones((G, 1), np.float32),
    }


def _shard_inputs(emissions, core):
    """Per-core scan-layout emissions: fwd half in order, bwd half reversed."""
    bsl = slice(core * BS, (core + 1) * BS)
    em4 = np.asarray(emissions[bsl], np.float32).reshape(G, BG, S, T)
    emf = em4[:, :, :HALF, :]                       # s = 0..255
    emb = em4[:, :, HALF:, :][:, :, ::-1, :]        # s = 511..256
    both = np.concatenate([emf, emb], axis=2)       # [G, BG, S, T]
    em_scan = both.transpose(0, 3, 2, 1).reshape(P, S * BG).astype(BF16)
    return {"em_scan": np.ascontiguousarray(em_scan)}


def _gold_host(emissions, tags, transitions, start_transitions, end_transitions):
    """Gold path score summed over the batch in float64 (pure tag indexing
    plus one emission gather)."""
    tg = np.asarray(tags).astype(np.int64)
    em = np.asarray(emissions)
    emit_sum = np.take_along_axis(em, tg[:, :, None], axis=2)[..., 0].sum(
        dtype=np.float64
    )
    trans_sum = np.asarray(transitions)[tg[:, :-1], tg[:, 1:]].sum(dtype=np.float64)
    start_sum = np.asarray(start_transitions)[tg[:, 0]].sum(dtype=np.float64)
    end_sum = np.asarray(end_transitions)[tg[:, -1]].sum(dtype=np.float64)
    return emit_sum + trans_sum + start_sum + end_sum


def _numpy_reference(emissions, tags, mask, transitions, start_transitions, end_transitions):
    """Slow numpy fallback, only used if mask is not all ones."""
    em = np.asarray(emissions, np.float64)
    tg = np.asarray(tags).astype(np.int64)
    mk = np.asarray(mask).astype(bool)
    Tm = np.asarray(transitions, np.float64)
    sv = np.asarray(start_transitions, np.float64)
    ev = np.asarray(end_transitions, np.float64)
    Bn, Sn, Tn = em.shape

    t0 = tg[:, 0]
    score = sv[t0] + np.take_along_axis(em[:, 0], t0[:, None], axis=1)[:, 0]
    maskf = mk[:, 1:].astype(np.float64)
    trans_sc = Tm[tg[:, :-1], tg[:, 1:]]
    emit_sc = np.take_along_axis(em[:, 1:], tg[:, 1:, None], axis=2)[..., 0]
    gold = score + ((trans_sc + emit_sc) * maskf).sum(axis=1)
    last_idx = mk.sum(axis=1).astype(np.int64) - 1
    last_tags = np.take_along_axis(tg, last_idx[:, None], axis=1)[:, 0]
    gold = gold + ev[last_tags]

    sc = sv[None, :] + em[:, 0]
    for s in range(1, Sn):
        nxt = sc[:, :, None] + Tm[None] + em[:, s][:, None, :]
        m = nxt.max(axis=1)
        nxt = m + np.log(np.exp(nxt - m[:, None, :]).sum(axis=1))
        sc = np.where(mk[:, s][:, None], nxt, sc)
    sc = sc + ev[None, :]
    m = sc.max(axis=1)
    fwd = m + np.log(np.exp(sc - m[:, None]).sum(axis=1))
    return np.array((fwd - gold).mean(), np.float32)


def kernel(emissions, tags, mask, transitions, start_transitions, end_transitions,
           _want_results=False, _trace=False):
    emissions = np.asarray(emissions)
    tags = np.asarray(tags)
    mask = np.asarray(mask)

    if not mask.all():
        return _numpy_reference(
            emissions, tags, mask, transitions, start_transitions, end_transitions
        )

    from concourse.bass_utils import run_bass_kernel_spmd

    nc = _get_graph()
    shared = _host_inputs(transitions, start_transitions, end_transitions)
    in_maps = []
    for c in range(NCORES):
        m = dict(shared)
        m.update(_shard_inputs(emissions, c))
        in_maps.append(m)

    res = run_bass_kernel_spmd(nc, in_maps, list(range(NCORES)), trace=_trace)

    gold = _gold_host(emissions, tags, transitions, start_transitions, end_transitions)
    tot_fwd = 0.0
    for c in range(NCORES):
        tot_fwd += float(np.asarray(res.results[c]["out"], np.float64)[0, 0])
    tot_fwd += B * S * MU
    loss = (tot_fwd - gold) / B
    if _want_results:
        return np.array(loss, np.float32), res
    return np.array(loss, np.float32)


# revision 10
# speedup vs baseline: 2.3202x; 1.0198x over previous
"""CRF loss (forward-algorithm partition function minus gold path score, batch mean)
on 8 Trainium2 NeuronCores, data-parallel over the batch dimension.

Layout / algorithm notes
------------------------
Per core shard: 512 batches = 4 groups x 128 batch-columns.
State tiles [128 part = (group, tag), 128 free = batch col] in bf16.

The partition function runs as a BIDIRECTIONAL exp-space scan meeting in the
middle, which halves the serial matmul->multiply dependency chain:

    fwd:  alpha_s = (Mblk^T  @ alpha_{s-1}) * xp_s      s = 1..255
    bwd:  beta_s  = (MblkT^T @ beta_{s+1})  * xp_s      s = 510..256
    Z_b  = sum_t alpha_255[t,b] * (M beta_256)[t,b]

with xp_s = exp(em_s - MU), Mblk = blockdiag(exp(T)), MblkT =
blockdiag(exp(T)^T).  MU = log(T)+1 cancels the mean per-step growth, so the
state drifts only O(sqrt(S)) nats per batch and needs NO rescaling inside
fp32/bf16 exponent range; logZ = log(Z_b) + S*MU exactly.

The two chains are independent, so the tensor engine interleaves fwd/bwd
matmuls while the vector engine interleaves the emission multiplies: the
per-step serial latency is paid only 256 times instead of 512.

The gold path score is pure integer indexing on tags (gathers + bincounts)
plus one emission gather; it is computed on host in float64 (the device keeps
the O(B*S*T^2) forward algorithm).
"""

import numpy as np
import ml_dtypes

B, S, T = 4096, 512, 32
NCORES = 8
BS = B // NCORES          # batches per core
G, BG = 4, 128            # groups x batch-columns (G*BG == BS)
P = 128
HALF = S // 2             # steps per chain
CS = [8, 24] + [32] * 7   # chunk sizes (steps); small first chunk = fast ramp
CO = np.cumsum([0] + CS).tolist()   # chunk start offsets
NCH = len(CS)
MU = float(np.log(T) + 1.0)

BF16 = ml_dtypes.bfloat16

_GRAPH = None


def _build_graph():
    from concourse import bacc, mybir, tile

    f32 = mybir.dt.float32
    bf16 = mybir.dt.bfloat16
    Af = mybir.ActivationFunctionType
    Op = mybir.AluOpType
    AX = mybir.AxisListType.X

    nc = bacc.Bacc(
        "TRN2",
        target_bir_lowering=False,
        debug=False,
        enable_asserts=False,
        num_devices=NCORES,
    )

    em_scan = nc.dram_tensor("em_scan", [P, S * BG], bf16, kind="ExternalInput")
    mblk_in = nc.dram_tensor("mblk", [P, P], bf16, kind="ExternalInput")
    mblkT_in = nc.dram_tensor("mblkT", [P, P], bf16, kind="ExternalInput")
    svec = nc.dram_tensor("svec", [P, 1], f32, kind="ExternalInput")   # exp(start)
    evec = nc.dram_tensor("evec", [P, 1], f32, kind="ExternalInput")   # exp(end)
    bones4 = nc.dram_tensor("bones4", [P, G], bf16, kind="ExternalInput")
    onesG = nc.dram_tensor("onesG", [G, 1], f32, kind="ExternalInput")
    out = nc.dram_tensor("out", [1, 1], f32, kind="ExternalOutput")

    em_ap = em_scan.ap()
    BWD0 = HALF * BG          # column offset of the backward stream

    with tile.TileContext(nc) as tc:
        with (
            tc.tile_pool(name="cpool", bufs=1) as cpool,
            tc.tile_pool(name="empf", bufs=3) as empf,
            tc.tile_pool(name="empb", bufs=3) as empb,
            tc.tile_pool(name="xpf", bufs=3) as xpfp,
            tc.tile_pool(name="xpb", bufs=3) as xpbp,
            tc.tile_pool(name="apool", bufs=3) as apool,
            tc.tile_pool(name="bpool", bufs=3) as bpool,
            tc.tile_pool(name="psf", bufs=2, space="PSUM") as psfp,
            tc.tile_pool(name="psb", bufs=2, space="PSUM") as psbp,
            tc.tile_pool(name="psx", bufs=1, space="PSUM") as psxp,
        ):
            # ---- constants (all exp-transformed on host) ----
            es_t = cpool.tile([P, 1], f32)
            nc.sync.dma_start(out=es_t[:], in_=svec.ap())
            ee_t = cpool.tile([P, 1], f32)
            nc.sync.dma_start(out=ee_t[:], in_=evec.ap())
            mblk_t = cpool.tile([P, P], bf16)
            nc.sync.dma_start(out=mblk_t[:], in_=mblk_in.ap())
            mblkT_t = cpool.tile([P, P], bf16)
            nc.sync.dma_start(out=mblkT_t[:], in_=mblkT_in.ap())
            bones4_t = cpool.tile([P, G], bf16)
            nc.sync.dma_start(out=bones4_t[:], in_=bones4.ap())
            onesG_t = cpool.tile([G, 1], f32)
            nc.sync.dma_start(out=onesG_t[:], in_=onesG.ap())

            negmu_t = cpool.tile([P, 1], f32)
            nc.vector.memset(negmu_t[:], -MU)

            # ---- emission chunk streams (double-buffered DMA + exp) ----
            def issue_chunk(c):
                lo, n = CO[c] * BG, CS[c] * BG
                emf_t = empf.tile([P, n], bf16, name="emf")
                nc.sync.dma_start(out=emf_t[:], in_=em_ap[:, lo : lo + n])
                xf_t = xpfp.tile([P, n], bf16, name="xpf")
                nc.scalar.activation(xf_t[:], emf_t[:], Af.Exp, bias=negmu_t[:])
                emb_t = empb.tile([P, n], bf16, name="emb")
                nc.sync.dma_start(out=emb_t[:], in_=em_ap[:, BWD0 + lo : BWD0 + lo + n])
                xb_t = xpbp.tile([P, n], bf16, name="xpb")
                nc.scalar.activation(xb_t[:], emb_t[:], Af.Exp, bias=negmu_t[:])
                return xf_t, xb_t

            # window -> (chunk, offset-in-chunk) map
            w2c = []
            for ci, n in enumerate(CS):
                w2c += [(ci, so) for so in range(n)]

            xf_t, xb_t = issue_chunk(0)
            pending = [issue_chunk(1), issue_chunk(2)]

            # ---- init both chains (window 0) ----
            alpha = apool.tile([P, BG], bf16, tag="alpha", name="alpha")
            nc.vector.tensor_scalar_mul(alpha[:], xf_t[:, 0:BG], es_t[:])
            beta = bpool.tile([P, BG], bf16, tag="beta", name="beta")
            nc.vector.tensor_scalar_mul(beta[:], xb_t[:, 0:BG], ee_t[:])

            # ---- main bidirectional scan: windows 1..HALF-1 ----
            for w in range(1, HALF):
                c, so = w2c[w]
                if so == 0:
                    xf_t, xb_t = pending.pop(0)
                    if c + 2 < NCH:
                        pending.append(issue_chunk(c + 2))

                psf = psfp.tile([P, BG], f32, tag="psf", name="psf")
                nc.tensor.matmul(psf[:], lhsT=mblk_t[:], rhs=alpha[:], start=True, stop=True)
                psb = psbp.tile([P, BG], f32, tag="psb", name="psb")
                nc.tensor.matmul(psb[:], lhsT=mblkT_t[:], rhs=beta[:], start=True, stop=True)

                alpha_new = apool.tile([P, BG], bf16, tag="alpha", name="alpha")
                nc.vector.tensor_tensor(
                    alpha_new[:], psf[:], xf_t[:, so * BG : (so + 1) * BG], Op.mult
                )
                alpha = alpha_new
                beta_new = bpool.tile([P, BG], bf16, tag="beta", name="beta")
                nc.vector.tensor_tensor(
                    beta_new[:], psb[:], xb_t[:, so * BG : (so + 1) * BG], Op.mult
                )
                beta = beta_new

            # ---- junction: Z = sum_t alpha_255 * (M beta_256) ----
            psj = psfp.tile([P, BG], f32, tag="psf", name="psj")
            nc.tensor.matmul(psj[:], lhsT=mblkT_t[:], rhs=beta[:], start=True, stop=True)
            zt = apool.tile([P, BG], bf16, tag="alpha", name="zt")
            nc.vector.tensor_tensor(zt[:], psj[:], alpha[:], Op.mult)

            gs = psxp.tile([G, BG], f32, tag="gs", name="gs")
            nc.tensor.matmul(gs[:], lhsT=bones4_t[:], rhs=zt[:], start=True, stop=True)
            lngs_t = cpool.tile([G, BG], f32)
            nc.scalar.activation(lngs_t[:], gs[:], Af.Ln)
            colsum_t = cpool.tile([G, 1], f32)
            nc.vector.reduce_sum(colsum_t[:], lngs_t[:], axis=AX)

            fin = psxp.tile([1, 1], f32, tag="fin", name="fin")
            nc.tensor.matmul(fin[:], lhsT=onesG_t[:], rhs=colsum_t[:], start=True, stop=True)
            outsb = cpool.tile([1, 1], f32)
            nc.vector.tensor_copy(outsb[:], fin[:])
            nc.sync.dma_start(out=out.ap(), in_=outsb[:])

    nc.compile()
    return nc


def _get_graph():
    global _GRAPH
    if _GRAPH is None:
        _GRAPH = _build_graph()
    return _GRAPH


def _host_inputs(transitions, start_transitions, end_transitions):
    """Constant / parameter-layout tensors shared by all cores (already
    exponentiated so the device preamble is DMA-only)."""
    Tm = np.asarray(transitions, np.float32)
    sv = np.asarray(start_transitions, np.float32)
    ev = np.asarray(end_transitions, np.float32)

    Mexp = np.exp(Tm).astype(BF16)
    MexpT = np.exp(Tm.T).astype(BF16)
    mblk = np.zeros((P, P), BF16)
    mblkT = np.zeros((P, P), BF16)
    for g in range(G):
        sl = slice(g * 32, (g + 1) * 32)
        mblk[sl, sl] = Mexp
        mblkT[sl, sl] = MexpT

    k = np.arange(P)
    bones4 = (np.arange(G)[None, :] == (k[:, None] // 32)).astype(BF16)  # [P, G]

    return {
        "mblk": mblk,
        "mblkT": mblkT,
        "svec": np.exp(np.tile(sv, G))[:, None].astype(np.float32),
        "evec": np.exp(np.tile(ev, G))[:, None].astype(np.float32),
        "bones4": np.ascontiguousarray(bones4),
        "onesG": np.ones((G, 1), np.float32),
    }


def _shard_inputs(emissions, core):
    """Per-core scan-layout emissions: fwd half in order, bwd half reversed."""
    bsl = slice(core * BS, (core + 1) * BS)
    em4 = np.asarray(emissions[bsl], np.float32).reshape(G, BG, S, T)
    emf = em4[:, :, :HALF, :]                       # s = 0..255
    emb = em4[:, :, HALF:, :][:, :, ::-1, :]        # s = 511..256
    both = np.concatenate([emf, emb], axis=2)       # [G, BG, S, T]
    em_scan = both.transpose(0, 3, 2, 1).reshape(P, S * BG).astype(BF16)
    return {"em_scan": np.ascontiguousarray(em_scan)}


def _gold_host(emissions, tags, transitions, start_transitions, end_transitions):
    """Gold path score summed over the batch in float64 (pure tag indexing
    plus one emission gather)."""
    tg = np.asarray(tags).astype(np.int64)
    em = np.asarray(emissions)
    emit_sum = np.take_along_axis(em, tg[:, :, None], axis=2)[..., 0].sum(
        dtype=np.float64
    )
    trans_sum = np.asarray(transitions)[tg[:, :-1], tg[:, 1:]].sum(dtype=np.float64)
    start_sum = np.asarray(start_transitions)[tg[:, 0]].sum(dtype=np.float64)
    end_sum = np.asarray(end_transitions)[tg[:, -1]].sum(dtype=np.float64)
    return emit_sum + trans_sum + start_sum + end_sum


def _numpy_reference(emissions, tags, mask, transitions, start_transitions, end_transitions):
    """Slow numpy fallback, only used if mask is not all ones."""
    em = np.asarray(emissions, np.float64)
    tg = np.asarray(tags).astype(np.int64)
    mk = np.asarray(mask).astype(bool)
    Tm = np.asarray(transitions, np.float64)
    sv = np.asarray(start_transitions, np.float64)
    ev = np.asarray(end_transitions, np.float64)
    Bn, Sn, Tn = em.shape

    t0 = tg[:, 0]
    score = sv[t0] + np.take_along_axis(em[:, 0], t0[:, None], axis=1)[:, 0]
    maskf = mk[:, 1:].astype(np.float64)
    trans_sc = Tm[tg[:, :-1], tg[:, 1:]]
    emit_sc = np.take_along_axis(em[:, 1:], tg[:, 1:, None], axis=2)[..., 0]
    gold = score + ((trans_sc + emit_sc) * maskf).sum(axis=1)
    last_idx = mk.sum(axis=1).astype(np.int64) - 1
    last_tags = np.take_along_axis(tg, last_idx[:, None], axis=1)[:, 0]
    gold = gold + ev[last_tags]

    sc = sv[None, :] + em[:, 0]
    for s in range(1, Sn):
        nxt = sc[:, :, None] + Tm[None] + em[:, s][:, None, :]
        m = nxt.max(axis=1)
        nxt = m + np.log(np.exp(nxt - m[:, None, :]).sum(axis=1))
        sc = np.where(mk[:, s][:, None], nxt, sc)
    sc = sc + ev[None, :]
    m = sc.max(axis=1)
    fwd = m + np.log(np.exp(sc - m[:, None]).sum(axis=1))
    return np.array((fwd - gold).mean(), np.float32)


def kernel(emissions, tags, mask, transitions, start_transitions, end_transitions,
           _want_results=False, _trace=False):
    emissions = np.asarray(emissions)
    tags = np.asarray(tags)
    mask = np.asarray(mask)

    if not mask.all():
        return _numpy_reference(
            emissions, tags, mask, transitions, start_transitions, end_transitions
        )

    from concourse.bass_utils import run_bass_kernel_spmd

    nc = _get_graph()
    shared = _host_inputs(transitions, start_transitions, end_transitions)
    in_maps = []
    for c in range(NCORES):
        m = dict(shared)
        m.update(_shard_inputs(emissions, c))
        in_maps.append(m)

    res = run_bass_kernel_spmd(nc, in_maps, list(range(NCORES)), trace=_trace)

    gold = _gold_host(emissions, tags, transitions, start_transitions, end_transitions)
    tot_fwd = 0.0
    for c in range(NCORES):
        tot_fwd += float(np.asarray(res.results[c]["out"], np.float64)[0, 0])
    tot_fwd += B * S * MU
    loss = (tot_fwd - gold) / B
    if _want_results:
        return np.array(loss, np.float32), res
    return np.array(loss, np.float32)


# revision 15
# speedup vs baseline: 2.3626x; 1.0183x over previous
"""CRF loss (forward-algorithm partition function minus gold path score, batch mean)
on 8 Trainium2 NeuronCores, data-parallel over the batch dimension.

Layout / algorithm notes
------------------------
Per core shard: 512 batches = 4 groups x 128 batch-columns.
State tiles [128 part = (group, tag), 128 free = batch col] in bf16.

The partition function runs as a BIDIRECTIONAL exp-space scan meeting in the
middle, which halves the serial matmul->multiply dependency chain:

    fwd:  alpha_s = (Mblk^T  @ alpha_{s-1}) * xp_s      s = 1..255
    bwd:  beta_s  = (MblkT^T @ beta_{s+1})  * xp_s      s = 510..256
    Z_b  = sum_t alpha_255[t,b] * (M beta_256)[t,b]

with xp_s = exp(em_s - MU), Mblk = blockdiag(exp(T)), MblkT =
blockdiag(exp(T)^T).  MU = log(T)+1 cancels the mean per-step growth, so the
state drifts only O(sqrt(S)) nats per batch and needs NO rescaling inside
fp32/bf16 exponent range; logZ = log(Z_b) + S*MU exactly.

The two chains are independent, so the tensor engine interleaves fwd/bwd
matmuls while the vector engine interleaves the emission multiplies: the
per-step serial latency is paid only 256 times instead of 512.

The gold path score is pure integer indexing on tags (gathers + bincounts)
plus one emission gather; it is computed on host in float64 (the device keeps
the O(B*S*T^2) forward algorithm).
"""

import numpy as np
import ml_dtypes

B, S, T = 4096, 512, 32
NCORES = 8
BS = B // NCORES          # batches per core
G, BG = 4, 128            # groups x batch-columns (G*BG == BS)
P = 128
HALF = S // 2             # steps per chain
CS = [8, 24] + [32] * 7   # chunk sizes (steps); small first chunk = fast ramp
CO = np.cumsum([0] + CS).tolist()   # chunk start offsets
NCH = len(CS)
MU = float(np.log(T) + 1.0)

BF16 = ml_dtypes.bfloat16

_GRAPH = None


def _build_graph():
    from concourse import bacc, mybir, tile

    f32 = mybir.dt.float32
    bf16 = mybir.dt.bfloat16
    Af = mybir.ActivationFunctionType
    Op = mybir.AluOpType
    AX = mybir.AxisListType.X

    nc = bacc.Bacc(
        "TRN2",
        target_bir_lowering=False,
        debug=False,
        enable_asserts=False,
        num_devices=NCORES,
    )

    em_scan = nc.dram_tensor("em_scan", [P, S * BG], bf16, kind="ExternalInput")
    mblks_in = nc.dram_tensor("mblks", [P, 2 * P], bf16, kind="ExternalInput")
    esee_in = nc.dram_tensor("esee", [P, 2], f32, kind="ExternalInput")  # exp(start), exp(end)
    bones4 = nc.dram_tensor("bones4", [P, G], bf16, kind="ExternalInput")
    out = nc.dram_tensor("out", [1, 1], f32, kind="ExternalOutput")

    em_ap = em_scan.ap()
    BWD0 = HALF * BG          # column offset of the backward stream

    with tile.TileContext(nc) as tc:
        with (
            tc.tile_pool(name="cpool", bufs=1) as cpool,
            tc.tile_pool(name="empf", bufs=3) as empf,
            tc.tile_pool(name="empb", bufs=3) as empb,
            tc.tile_pool(name="xpf", bufs=3) as xpfp,
            tc.tile_pool(name="xpb", bufs=3) as xpbp,
            tc.tile_pool(name="apool", bufs=3) as apool,
            tc.tile_pool(name="bpool", bufs=3) as bpool,
            tc.tile_pool(name="psf", bufs=2, space="PSUM") as psfp,
            tc.tile_pool(name="psb", bufs=2, space="PSUM") as psbp,
            tc.tile_pool(name="psx", bufs=1, space="PSUM") as psxp,
        ):
            # ---- warm the Exp table while the first DMAs are in flight ----
            negmu_t = cpool.tile([P, 1], f32)
            nc.vector.memset(negmu_t[:], -MU)
            warm_t = cpool.tile([P, 1], f32)
            nc.scalar.activation(warm_t[:], negmu_t[:], Af.Exp)

            # ---- emission chunk streams (double-buffered DMA + exp) ----
            def issue_chunk(c):
                lo, n = CO[c] * BG, CS[c] * BG
                emf_t = empf.tile([P, n], bf16, name="emf")
                nc.sync.dma_start(out=emf_t[:], in_=em_ap[:, lo : lo + n])
                xf_t = xpfp.tile([P, n], bf16, name="xpf")
                nc.scalar.activation(xf_t[:], emf_t[:], Af.Exp, bias=negmu_t[:])
                emb_t = empb.tile([P, n], bf16, name="emb")
                nc.sync.dma_start(out=emb_t[:], in_=em_ap[:, BWD0 + lo : BWD0 + lo + n])
                xb_t = xpbp.tile([P, n], bf16, name="xpb")
                nc.scalar.activation(xb_t[:], emb_t[:], Af.Exp, bias=negmu_t[:])
                return xf_t, xb_t

            # window -> (chunk, offset-in-chunk) map
            w2c = []
            for ci, n in enumerate(CS):
                w2c += [(ci, so) for so in range(n)]

            xf_t, xb_t = issue_chunk(0)

            # ---- constants (issued after chunk 0 so its DMA lands first) ----
            esee_t = cpool.tile([P, 2], f32)
            nc.sync.dma_start(out=esee_t[:], in_=esee_in.ap())
            es_t, ee_t = esee_t[:, 0:1], esee_t[:, 1:2]
            mblks_t = cpool.tile([P, 2 * P], bf16)
            nc.sync.dma_start(out=mblks_t[:], in_=mblks_in.ap())
            mblk_t, mblkT_t = mblks_t[:, 0:P], mblks_t[:, P : 2 * P]
            bones4_t = cpool.tile([P, G], bf16)
            nc.sync.dma_start(out=bones4_t[:], in_=bones4.ap())
            onesG_t = cpool.tile([G, 1], f32)
            nc.vector.memset(onesG_t[:], 1.0)

            pending = [issue_chunk(1), issue_chunk(2)]

            # ---- init both chains (window 0) ----
            alpha = apool.tile([P, BG], bf16, tag="alpha", name="alpha")
            nc.vector.tensor_scalar_mul(alpha[:], xf_t[:, 0:BG], es_t)
            beta = bpool.tile([P, BG], bf16, tag="beta", name="beta")
            nc.vector.tensor_scalar_mul(beta[:], xb_t[:, 0:BG], ee_t)

            # ---- main bidirectional scan: windows 1..HALF-1 ----
            for w in range(1, HALF):
                c, so = w2c[w]
                if so == 0:
                    xf_t, xb_t = pending.pop(0)
                    if c + 2 < NCH:
                        pending.append(issue_chunk(c + 2))

                psf = psfp.tile([P, BG], f32, tag="psf", name="psf")
                nc.tensor.matmul(psf[:], lhsT=mblk_t, rhs=alpha[:], start=True, stop=True)
                psb = psbp.tile([P, BG], f32, tag="psb", name="psb")
                nc.tensor.matmul(psb[:], lhsT=mblkT_t, rhs=beta[:], start=True, stop=True)

                alpha_new = apool.tile([P, BG], bf16, tag="alpha", name="alpha")
                nc.vector.tensor_tensor(
                    alpha_new[:], psf[:], xf_t[:, so * BG : (so + 1) * BG], Op.mult
                )
                alpha = alpha_new
                beta_new = bpool.tile([P, BG], bf16, tag="beta", name="beta")
                nc.vector.tensor_tensor(
                    beta_new[:], psb[:], xb_t[:, so * BG : (so + 1) * BG], Op.mult
                )
                beta = beta_new

            # ---- junction: Z = sum_t alpha_255 * (M beta_256) ----
            psj = psfp.tile([P, BG], f32, tag="psf", name="psj")
            nc.tensor.matmul(psj[:], lhsT=mblkT_t, rhs=beta[:], start=True, stop=True)
            zt = apool.tile([P, BG], bf16, tag="alpha", name="zt")
            nc.vector.tensor_tensor(zt[:], psj[:], alpha[:], Op.mult)

            gs = psxp.tile([G, BG], f32, tag="gs", name="gs")
            nc.tensor.matmul(gs[:], lhsT=bones4_t[:], rhs=zt[:], start=True, stop=True)
            lngs_t = cpool.tile([G, BG], f32)
            nc.scalar.activation(lngs_t[:], gs[:], Af.Ln)
            colsum_t = cpool.tile([G, 1], f32)
            nc.vector.reduce_sum(colsum_t[:], lngs_t[:], axis=AX)

            fin = psxp.tile([1, 1], f32, tag="fin", name="fin")
            nc.tensor.matmul(fin[:], lhsT=onesG_t[:], rhs=colsum_t[:], start=True, stop=True)
            outsb = cpool.tile([1, 1], f32)
            nc.vector.tensor_copy(outsb[:], fin[:])
            nc.sync.dma_start(out=out.ap(), in_=outsb[:])

    nc.compile()
    return nc


def _get_graph():
    global _GRAPH
    if _GRAPH is None:
        _GRAPH = _build_graph()
    return _GRAPH


def _host_inputs(transitions, start_transitions, end_transitions):
    """Constant / parameter-layout tensors shared by all cores (already
    exponentiated so the device preamble is DMA-only)."""
    Tm = np.asarray(transitions, np.float32)
    sv = np.asarray(start_transitions, np.float32)
    ev = np.asarray(end_transitions, np.float32)

    Mexp = np.exp(Tm).astype(BF16)
    MexpT = np.exp(Tm.T).astype(BF16)
    mblks = np.zeros((P, 2 * P), BF16)
    for g in range(G):
        sl = slice(g * 32, (g + 1) * 32)
        mblks[sl, sl] = Mexp
        mblks[sl, P + g * 32 : P + (g + 1) * 32] = MexpT

    esee = np.stack(
        [np.exp(np.tile(sv, G)), np.exp(np.tile(ev, G))], axis=1
    ).astype(np.float32)

    k = np.arange(P)
    bones4 = (np.arange(G)[None, :] == (k[:, None] // 32)).astype(BF16)  # [P, G]

    return {
        "mblks": mblks,
        "esee": np.ascontiguousarray(esee),
        "bones4": np.ascontiguousarray(bones4),
    }


def _shard_inputs(emissions, core):
    """Per-core scan-layout emissions: fwd half in order, bwd half reversed."""
    bsl = slice(core * BS, (core + 1) * BS)
    em4 = np.asarray(emissions[bsl], np.float32).reshape(G, BG, S, T)
    emf = em4[:, :, :HALF, :]                       # s = 0..255
    emb = em4[:, :, HALF:, :][:, :, ::-1, :]        # s = 511..256
    both = np.concatenate([emf, emb], axis=2)       # [G, BG, S, T]
    em_scan = both.transpose(0, 3, 2, 1).reshape(P, S * BG).astype(BF16)
    return {"em_scan": np.ascontiguousarray(em_scan)}


def _gold_host(emissions, tags, transitions, start_transitions, end_transitions):
    """Gold path score summed over the batch in float64 (pure tag indexing
    plus one emission gather)."""
    tg = np.asarray(tags).astype(np.int64)
    em = np.asarray(emissions)
    emit_sum = np.take_along_axis(em, tg[:, :, None], axis=2)[..., 0].sum(
        dtype=np.float64
    )
    trans_sum = np.asarray(transitions)[tg[:, :-1], tg[:, 1:]].sum(dtype=np.float64)
    start_sum = np.asarray(start_transitions)[tg[:, 0]].sum(dtype=np.float64)
    end_sum = np.asarray(end_transitions)[tg[:, -1]].sum(dtype=np.float64)
    return emit_sum + trans_sum + start_sum + end_sum


def _numpy_reference(emissions, tags, mask, transitions, start_transitions, end_transitions):
    """Slow numpy fallback, only used if mask is not all ones."""
    em = np.asarray(emissions, np.float64)
    tg = np.asarray(tags).astype(np.int64)
    mk = np.asarray(mask).astype(bool)
    Tm = np.asarray(transitions, np.float64)
    sv = np.asarray(start_transitions, np.float64)
    ev = np.asarray(end_transitions, np.float64)
    Bn, Sn, Tn = em.shape

    t0 = tg[:, 0]
    score = sv[t0] + np.take_along_axis(em[:, 0], t0[:, None], axis=1)[:, 0]
    maskf = mk[:, 1:].astype(np.float64)
    trans_sc = Tm[tg[:, :-1], tg[:, 1:]]
    emit_sc = np.take_along_axis(em[:, 1:], tg[:, 1:, None], axis=2)[..., 0]
    gold = score + ((trans_sc + emit_sc) * maskf).sum(axis=1)
    last_idx = mk.sum(axis=1).astype(np.int64) - 1
    last_tags = np.take_along_axis(tg, last_idx[:, None], axis=1)[:, 0]
    gold = gold + ev[last_tags]

    sc = sv[None, :] + em[:, 0]
    for s in range(1, Sn):
        nxt = sc[:, :, None] + Tm[None] + em[:, s][:, None, :]
        m = nxt.max(axis=1)
        nxt = m + np.log(np.exp(nxt - m[:, None, :]).sum(axis=1))
        sc = np.where(mk[:, s][:, None], nxt, sc)
    sc = sc + ev[None, :]
    m = sc.max(axis=1)
    fwd = m + np.log(np.exp(sc - m[:, None]).sum(axis=1))
    return np.array((fwd - gold).mean(), np.float32)


def kernel(emissions, tags, mask, transitions, start_transitions, end_transitions,
           _want_results=False, _trace=False):
    emissions = np.asarray(emissions)
    tags = np.asarray(tags)
    mask = np.asarray(mask)

    if not mask.all():
        return _numpy_reference(
            emissions, tags, mask, transitions, start_transitions, end_transitions
        )

    from concourse.bass_utils import run_bass_kernel_spmd

    nc = _get_graph()
    shared = _host_inputs(transitions, start_transitions, end_transitions)
    in_maps = []
    for c in range(NCORES):
        m = dict(shared)
        m.update(_shard_inputs(emissions, c))
        in_maps.append(m)

    res = run_bass_kernel_spmd(nc, in_maps, list(range(NCORES)), trace=_trace)

    gold = _gold_host(emissions, tags, transitions, start_transitions, end_transitions)
    tot_fwd = 0.0
    for c in range(NCORES):
        tot_fwd += float(np.asarray(res.results[c]["out"], np.float64)[0, 0])
    tot_fwd += B * S * MU
    loss = (tot_fwd - gold) / B
    if _want_results:
        return np.array(loss, np.float32), res
    return np.array(loss, np.float32)
